# revision 8
# baseline (speedup 1.0000x reference)
"""Trainium2 Bass kernel for nn_GRUModel: GRU(I=3, H=50) over [B=4096, T=512],
followed by a linear head to one output per batch element.

Strategy (8 cores data-parallel, B=512 per core; 2 decoupled batch streams
of 256 per core so ACT/DVE/PE/GpSimd overlap across streams):
  - Layout per stream: rhs ring [K=68, nslot*256] fp16: h rows 0-49,
    x_t rows 50-52, ones row 53; x_t dup rows 64-66, ones row 67 (so the
    g-gate matmul can use PE row-groups 2-3 and run concurrent with MM1).
  - Per step per stream:
      MM1  (W1 [54,128], z 0-49 | r 64-113; rhs rows 0-53)  -> ps1
      MM2g (W2g [4,128], g cols 0-49; rhs rows 64-67, tile_position (64,0),
            start)  -> ps2   [runs concurrent with MM1: disjoint row groups]
      MM2p (W2p [54,128], p cols 64-113; rhs rows 0-53, accum) -> ps2
      sigmoid(ps1[0:114]) -> zr fp16
      v = zr[64:114] * ps2[64:114]          (DVE, = r * p)
      MM3  (I50, rhs=v, accum stop) -> ps2[0:50] += v   (u = g + r*p on PE)
      tanh(ps2[0:50]) -> n fp16
      s = h - n (DVE); q = z * s (DVE); h' = n + q -> ring slot t+1 (GpSimd)
  - final: out = W_fc @ h_T + b_fc per stream via [54,1] matmuls.
"""

import numpy as np
from contextlib import ExitStack

H = 50
I = 3
B_FULL = 4096
T_FULL = 512
NCORES = 8
B = B_FULL // NCORES  # 512 batch per core
NS = 2                # batch streams per core
BS = B // NS          # 256 batch per stream
K = 54                # matmul contraction rows: h 0-49, x 50-52, ones 53
KR = 68               # rhs tile rows incl. x-dup block at 64-67
M = 128               # weight cols (z|r and g|p at 0-49 / 64-113, zero pad)
NSLOT = 64            # rhs ring slots
TC = 32               # timesteps per x DMA chunk
GPS_HPRIME = True     # compute h' on GpSimd instead of DVE

_prog_cache = {}


def _host_weights(W_ih, W_hh, b_ih, b_hh, W_fc, b_fc):
    """Stationary lhsT matrices (fp16). Rows: h 0-49, x 50-52, ones 53."""
    f32 = np.float32
    W1 = np.zeros((K, M), f32)  # cols [z | pad | r]
    W1[0:H, 0:50] = W_hh[H : 2 * H].T
    W1[H : H + I, 0:50] = W_ih[H : 2 * H].T
    W1[K - 1, 0:50] = b_ih[H : 2 * H] + b_hh[H : 2 * H]
    W1[0:H, 64:114] = W_hh[0:H].T
    W1[H : H + I, 64:114] = W_ih[0:H].T
    W1[K - 1, 64:114] = b_ih[0:H] + b_hh[0:H]
    W2g = np.zeros((4, M), f32)  # rows [x;1], cols [g | 0]
    W2g[0:I, 0:50] = W_ih[2 * H :].T
    W2g[3, 0:50] = b_ih[2 * H :]
    W2p = np.zeros((K, M), f32)  # cols [0 | p]
    W2p[0:H, 64:114] = W_hh[2 * H :].T
    W2p[K - 1, 64:114] = b_hh[2 * H :]
    I50 = np.zeros((H, M), f32)
    I50[np.arange(H), np.arange(H)] = 1.0
    Wfc = np.zeros((K, 1), f32)
    Wfc[0:H, 0] = W_fc[0]
    Wfc[K - 1, 0] = b_fc[0]
    f16 = np.float16
    return (
        W1.astype(f16),
        W2g.astype(f16),
        W2p.astype(f16),
        I50.astype(f16),
        Wfc.astype(f16),
    )


def build_program(T=T_FULL, num_devices=NCORES):
    """Emit the per-core bass program (identical across cores)."""
    import concourse.bass as bass
    import concourse.tile as tile
    from concourse import bacc, mybir

    f16 = mybir.dt.float16
    f32 = mybir.dt.float32
    AF = mybir.ActivationFunctionType

    nc = bacc.Bacc(
        "TRN2", target_bir_lowering=False, debug=False, num_devices=num_devices
    )
    xts = [
        nc.dram_tensor(f"xt{s}", [T, I, BS], f16, kind="ExternalInput")
        for s in range(NS)
    ]
    w1 = nc.dram_tensor("w1", [K, M], f16, kind="ExternalInput")
    w2g = nc.dram_tensor("w2g", [4, M], f16, kind="ExternalInput")
    w2p = nc.dram_tensor("w2p", [K, M], f16, kind="ExternalInput")
    wi = nc.dram_tensor("wi", [H, M], f16, kind="ExternalInput")
    wfc = nc.dram_tensor("wfc", [K, 1], f16, kind="ExternalInput")
    out = nc.dram_tensor("out", [1, B], f32, kind="ExternalOutput")

    nchunk = (T + TC - 1) // TC

    with tile.TileContext(nc) as tc, ExitStack() as ctx:
        const = ctx.enter_context(tc.tile_pool(name="const", bufs=1))
        psum = ctx.enter_context(tc.tile_pool(name="psum", bufs=1, space="PSUM"))
        psumf = ctx.enter_context(tc.tile_pool(name="psumf", bufs=1, space="PSUM"))
        work = ctx.enter_context(tc.tile_pool(name="work", bufs=2))

        w1_sb = const.tile([K, M], f16, tag="w1")
        w2g_big = const.tile([68, M], f16, tag="w2g")  # rows 64-67 hold W2g
        w2g_sb = w2g_big[64:68, :]
        w2p_sb = const.tile([K, M], f16, tag="w2p")
        wi_sb = const.tile([H, M], f16, tag="wi")
        wfc_sb = const.tile([K, 1], f16, tag="wfc")
        rhs = [
            const.tile([KR, NSLOT * BS], f16, tag=f"rhs{s}", name=f"rhs{s}")
            for s in range(NS)
        ]
        out_sb = const.tile([1, B], f32, tag="out_sb")

        nc.sync.dma_start(w1_sb[:], w1.ap())
        nc.sync.dma_start(w2g_sb, w2g.ap())
        nc.sync.dma_start(w2p_sb[:], w2p.ap())
        nc.sync.dma_start(wi_sb[:], wi.ap())
        nc.sync.dma_start(wfc_sb[:], wfc.ap())

        for s in range(NS):
            # ones everywhere (rows 53/67 persist; x rows overwritten by DMA),
            # h0 = 0 in slot 0
            nc.gpsimd.memset(rhs[s][0:KR, :], 1.0)
            nc.gpsimd.memset(rhs[s][0:H, 0:BS], 0.0)

        def dma_x_chunk(s, c):
            t0 = c * TC
            tcnt = min(TC, T - t0)
            if tcnt <= 0:
                return
            slot0 = (t0 % NSLOT) * BS
            for xrow in (H, 64):
                src = xts[s].ap()[t0 : t0 + tcnt].rearrange("t i b -> i t b")
                dst = rhs[s][xrow : xrow + I, slot0 : slot0 + tcnt * BS].rearrange(
                    "p (t b) -> p t b", t=tcnt
                )
                nc.sync.dma_start(dst, src)

        for s in range(NS):
            dma_x_chunk(s, 0)
            dma_x_chunk(s, 1)

        for t in range(T):
            slot = t % NSLOT
            nxt = ((t + 1) % NSLOT) * BS
            if t % TC == 0 and t // TC + 2 < nchunk:
                for s in range(NS):
                    dma_x_chunk(s, t // TC + 2)
            for s in range(NS):
                rhs_t = rhs[s][0:K, slot * BS : (slot + 1) * BS]
                rhs_x2 = rhs[s][64:68, slot * BS : (slot + 1) * BS]
                ps1 = psum.tile([M, BS], f32, tag=f"ps1{s}")
                nc.tensor.matmul(ps1[:], w1_sb[:], rhs_t, start=True, stop=True)
                ps2 = psum.tile([M, BS], f32, tag=f"ps2{s}")
                nc.tensor.matmul(
                    ps2[:], w2g_sb, rhs_x2, start=True, stop=False,
                    tile_position=(64, 0),
                )
                nc.tensor.matmul(ps2[:], w2p_sb[:], rhs_t, start=False, stop=False)
                zr = work.tile([114, BS], f16, tag=f"zr{s}")
                nc.scalar.activation(zr[:], ps1[0:114, :], AF.Sigmoid)
                v = work.tile([H, BS], f16, tag=f"v{s}")
                nc.vector.tensor_mul(v[:], zr[64:114, :], ps2[64:114, :])
                nc.tensor.matmul(ps2[:], wi_sb[:], v[:], start=False, stop=True)
                n = work.tile([H, BS], f16, tag=f"n{s}")
                nc.scalar.activation(n[:], ps2[0:H, :], AF.Tanh)
                sb = work.tile([H, BS], f16, tag=f"s{s}")
                nc.vector.tensor_sub(
                    sb[:], rhs[s][0:H, slot * BS : (slot + 1) * BS], n[:]
                )
                q = work.tile([H, BS], f16, tag=f"q{s}")
                nc.vector.tensor_mul(q[:], zr[0:H, :], sb[:])
                if GPS_HPRIME:
                    nc.gpsimd.tensor_add(
                        rhs[s][0:H, nxt : nxt + BS], n[:], q[:]
                    )
                else:
                    nc.vector.tensor_add(rhs[s][0:H, nxt : nxt + BS], n[:], q[:])

        fslot = (T % NSLOT) * BS
        for s in range(NS):
            psf = psumf.tile([1, BS], f32, tag=f"psf{s}")
            nc.tensor.matmul(
                psf[:], wfc_sb[:], rhs[s][0:K, fslot : fslot + BS], start=True, stop=True
            )
            nc.scalar.copy(out_sb[0:1, s * BS : (s + 1) * BS], psf[:])
        nc.sync.dma_start(out.ap(), out_sb[:])

    nc.compile()
    return nc


def _prepare_in_maps(inputs, T=T_FULL):
    x = np.asarray(inputs["x"], dtype=np.float32)
    W1, W2g, W2p, I50, Wfc = _host_weights(
        np.asarray(inputs["W_ih"], np.float32),
        np.asarray(inputs["W_hh"], np.float32),
        np.asarray(inputs["b_ih"], np.float32),
        np.asarray(inputs["b_hh"], np.float32),
        np.asarray(inputs["W_fc"], np.float32),
        np.asarray(inputs["b_fc"], np.float32),
    )
    in_maps = []
    for c in range(NCORES):
        xs = x[c * B : (c + 1) * B, :T]  # [B, T, I]
        im = {"w1": W1, "w2g": W2g, "w2p": W2p, "wi": I50, "wfc": Wfc}
        for s in range(NS):
            xss = xs[s * BS : (s + 1) * BS]  # [BS, T, I]
            im[f"xt{s}"] = np.ascontiguousarray(xss.transpose(1, 2, 0)).astype(
                np.float16
            )
        in_maps.append(im)
    return in_maps


def kernel(x, W_ih, W_hh, b_ih, b_hh, W_fc, b_fc):
    from concourse.bass_utils import run_bass_kernel_spmd

    inputs = dict(x=x, W_ih=W_ih, W_hh=W_hh, b_ih=b_ih, b_hh=b_hh, W_fc=W_fc, b_fc=b_fc)
    if "prog" not in _prog_cache:
        _prog_cache["prog"] = build_program()
    nc = _prog_cache["prog"]
    in_maps = _prepare_in_maps(inputs)
    res = run_bass_kernel_spmd(nc, in_maps, core_ids=list(range(NCORES)))
    outs = [res.results[c]["out"].reshape(B) for c in range(NCORES)]
    return np.concatenate(outs).astype(np.float32)


# revision 9
# speedup vs baseline: 1.0176x; 1.0176x over previous
"""Trainium2 Bass kernel for nn_GRUModel: GRU(I=3, H=50) over [B=4096, T=512],
followed by a linear head to one output per batch element.

Strategy (8 cores data-parallel, B=512 per core; 2 decoupled batch streams
of 256 per core so ACT/DVE/PE overlap across streams):
  - Layout per stream: rhs ring [K=54, nslot*256] fp16: h rows 0-49,
    x_t rows 50-52, ones row 53 (folds all biases + input projection into
    the recurrent matmuls; matmul cost depends only on N).
  - Per step per stream:
      MM1 (W1 [54,128], cols z 0-49 | r 64-113) -> ps1 = z/r pre-activations
      MM2 (W2 [54,128], cols g 0-49 | p 64-113, start) -> ps2 = [gi_n+b_in ; W_hn h+b_hn]
      sigmoid(ps1[0:114]) -> zr fp16
      v = zr[64:114] * ps2[64:114]         (DVE, = r * p)
      MM3 (I50, rhs=v, accumulate stop) -> ps2[0:50] += v  (u = g + r*p on PE)
      tanh(ps2[0:50]) -> n fp16
      s = h - n; q = z * s; h' = n + q -> rhs ring slot t+1  (DVE 2x fp16)
  - final: out = W_fc @ h_T + b_fc per stream via [54,1] matmuls.
"""

import numpy as np
from contextlib import ExitStack

H = 50
I = 3
B_FULL = 4096
T_FULL = 512
NCORES = 8
B = B_FULL // NCORES  # 512 batch per core
NS = 2                # batch streams per core
BS = B // NS          # 256 batch per stream
K = 54                # rhs rows: h 0-49, x 50-52, ones 53
M = 128               # weight cols (z|r and g|p at 0-49 / 64-113, zero pad)
NSLOT = 64            # rhs ring slots
TC = 32               # timesteps per x DMA chunk

_prog_cache = {}


def _host_weights(W_ih, W_hh, b_ih, b_hh, W_fc, b_fc):
    """Stationary lhsT matrices (fp16). Rows: h 0-49, x 50-52, ones 53."""
    f32 = np.float32
    W1 = np.zeros((K, M), f32)  # cols [z | pad | r]
    # z gate (cols 0-49)
    W1[0:H, 0:50] = W_hh[H : 2 * H].T
    W1[H : H + I, 0:50] = W_ih[H : 2 * H].T
    W1[K - 1, 0:50] = b_ih[H : 2 * H] + b_hh[H : 2 * H]
    # r gate (cols 64-113)
    W1[0:H, 64:114] = W_hh[0:H].T
    W1[H : H + I, 64:114] = W_ih[0:H].T
    W1[K - 1, 64:114] = b_ih[0:H] + b_hh[0:H]
    W2 = np.zeros((K, M), f32)  # cols [g | pad | p]
    # g = x-part of n gate (cols 0-49)
    W2[H : H + I, 0:50] = W_ih[2 * H :].T
    W2[K - 1, 0:50] = b_ih[2 * H :]
    # p = h-part of n gate (cols 64-113)
    W2[0:H, 64:114] = W_hh[2 * H :].T
    W2[K - 1, 64:114] = b_hh[2 * H :]
    I50 = np.zeros((H, M), f32)
    I50[np.arange(H), np.arange(H)] = 1.0
    Wfc = np.zeros((K, 1), f32)
    Wfc[0:H, 0] = W_fc[0]
    Wfc[K - 1, 0] = b_fc[0]
    f16 = np.float16
    return W1.astype(f16), W2.astype(f16), I50.astype(f16), Wfc.astype(f16)


def build_program(T=T_FULL, num_devices=NCORES):
    """Emit the per-core bass program (identical across cores)."""
    import concourse.bass as bass
    import concourse.tile as tile
    from concourse import bacc, mybir

    f16 = mybir.dt.float16
    f32 = mybir.dt.float32
    AF = mybir.ActivationFunctionType

    nc = bacc.Bacc(
        "TRN2", target_bir_lowering=False, debug=False, num_devices=num_devices
    )
    xts = [
        nc.dram_tensor(f"xt{s}", [T, I, BS], f16, kind="ExternalInput")
        for s in range(NS)
    ]
    w1 = nc.dram_tensor("w1", [K, M], f16, kind="ExternalInput")
    w2 = nc.dram_tensor("w2", [K, M], f16, kind="ExternalInput")
    wi = nc.dram_tensor("wi", [H, M], f16, kind="ExternalInput")
    wfc = nc.dram_tensor("wfc", [K, 1], f16, kind="ExternalInput")
    out = nc.dram_tensor("out", [1, B], f32, kind="ExternalOutput")

    nchunk = (T + TC - 1) // TC

    with tile.TileContext(nc) as tc, ExitStack() as ctx:
        const = ctx.enter_context(tc.tile_pool(name="const", bufs=1))
        psum = ctx.enter_context(tc.tile_pool(name="psum", bufs=1, space="PSUM"))
        psumf = ctx.enter_context(tc.tile_pool(name="psumf", bufs=1, space="PSUM"))
        work = ctx.enter_context(tc.tile_pool(name="work", bufs=3))

        w1_sb = const.tile([K, M], f16, tag="w1")
        w2_sb = const.tile([K, M], f16, tag="w2")
        wi_sb = const.tile([H, M], f16, tag="wi")
        wfc_sb = const.tile([K, 1], f16, tag="wfc")
        rhs = [
            const.tile([K, NSLOT * BS], f16, tag=f"rhs{s}", name=f"rhs{s}")
            for s in range(NS)
        ]
        out_sb = const.tile([1, B], f32, tag="out_sb")

        nc.sync.dma_start(w1_sb[:], w1.ap())
        nc.sync.dma_start(w2_sb[:], w2.ap())
        nc.sync.dma_start(wi_sb[:], wi.ap())
        nc.sync.dma_start(wfc_sb[:], wfc.ap())

        for s in range(NS):
            # ones everywhere (row 53 persists; x rows overwritten by DMA),
            # h0 = 0 in slot 0
            nc.gpsimd.memset(rhs[s][0:K, :], 1.0)
            nc.gpsimd.memset(rhs[s][0:H, 0:BS], 0.0)

        def dma_x_chunk(s, c):
            t0 = c * TC
            tcnt = min(TC, T - t0)
            if tcnt <= 0:
                return
            slot0 = (t0 % NSLOT) * BS
            src = xts[s].ap()[t0 : t0 + tcnt].rearrange("t i b -> i t b")
            dst = rhs[s][H : H + I, slot0 : slot0 + tcnt * BS].rearrange(
                "p (t b) -> p t b", t=tcnt
            )
            nc.sync.dma_start(dst, src)

        for s in range(NS):
            dma_x_chunk(s, 0)
            dma_x_chunk(s, 1)

        for t in range(T):
            slot = t % NSLOT
            nxt = ((t + 1) % NSLOT) * BS
            for s in range(NS):
                if t % TC == 0 and t // TC + 2 < nchunk and s == 0:
                    dma_x_chunk(0, t // TC + 2)
                    dma_x_chunk(1, t // TC + 2)
                rhs_t = rhs[s][0:K, slot * BS : (slot + 1) * BS]
                ps1 = psum.tile([M, BS], f32, tag=f"ps1{s}")
                nc.tensor.matmul(ps1[:], w1_sb[:], rhs_t, start=True, stop=True)
                ps2 = psum.tile([M, BS], f32, tag=f"ps2{s}")
                nc.tensor.matmul(ps2[:], w2_sb[:], rhs_t, start=True, stop=False)
                zr = work.tile([114, BS], f16, tag=f"zr{s}")
                nc.scalar.activation(zr[:], ps1[0:114, :], AF.Sigmoid)
                v = work.tile([H, BS], f16, tag=f"v{s}")
                nc.vector.tensor_mul(v[:], zr[64:114, :], ps2[64:114, :])
                nc.tensor.matmul(ps2[:], wi_sb[:], v[:], start=False, stop=True)
                n = work.tile([H, BS], f16, tag=f"n{s}")
                nc.scalar.activation(n[:], ps2[0:H, :], AF.Tanh)
                sb = work.tile([H, BS], f16, tag=f"s{s}")
                nc.vector.tensor_sub(
                    sb[:], rhs[s][0:H, slot * BS : (slot + 1) * BS], n[:]
                )
                q = work.tile([H, BS], f16, tag=f"q{s}")
                nc.vector.tensor_mul(q[:], zr[0:H, :], sb[:])
                nc.gpsimd.tensor_add(rhs[s][0:H, nxt : nxt + BS], n[:], q[:])

        fslot = (T % NSLOT) * BS
        for s in range(NS):
            psf = psumf.tile([1, BS], f32, tag=f"psf{s}")
            nc.tensor.matmul(
                psf[:], wfc_sb[:], rhs[s][0:K, fslot : fslot + BS], start=True, stop=True
            )
            nc.scalar.copy(out_sb[0:1, s * BS : (s + 1) * BS], psf[:])
        nc.sync.dma_start(out.ap(), out_sb[:])

    nc.compile()
    return nc


def _prepare_in_maps(inputs, T=T_FULL):
    x = np.asarray(inputs["x"], dtype=np.float32)
    W1, W2, I50, Wfc = _host_weights(
        np.asarray(inputs["W_ih"], np.float32),
        np.asarray(inputs["W_hh"], np.float32),
        np.asarray(inputs["b_ih"], np.float32),
        np.asarray(inputs["b_hh"], np.float32),
        np.asarray(inputs["W_fc"], np.float32),
        np.asarray(inputs["b_fc"], np.float32),
    )
    in_maps = []
    for c in range(NCORES):
        xs = x[c * B : (c + 1) * B, :T]  # [B, T, I]
        im = {"w1": W1, "w2": W2, "wi": I50, "wfc": Wfc}
        for s in range(NS):
            xss = xs[s * BS : (s + 1) * BS]  # [BS, T, I]
            im[f"xt{s}"] = np.ascontiguousarray(xss.transpose(1, 2, 0)).astype(
                np.float16
            )
        in_maps.append(im)
    return in_maps


def kernel(x, W_ih, W_hh, b_ih, b_hh, W_fc, b_fc):
    from concourse.bass_utils import run_bass_kernel_spmd

    inputs = dict(x=x, W_ih=W_ih, W_hh=W_hh, b_ih=b_ih, b_hh=b_hh, W_fc=W_fc, b_fc=b_fc)
    if "prog" not in _prog_cache:
        _prog_cache["prog"] = build_program()
    nc = _prog_cache["prog"]
    in_maps = _prepare_in_maps(inputs)
    res = run_bass_kernel_spmd(nc, in_maps, core_ids=list(range(NCORES)))
    outs = [res.results[c]["out"].reshape(B) for c in range(NCORES)]
    return np.concatenate(outs).astype(np.float32)
